# revision 1
# baseline (speedup 1.0000x reference)
"""ClusterKLLoss Trainium2 kernel (8 NeuronCores, data-parallel over rows of c_i).

Math (derived from the reference):
  loss = CE(logits, arange(B), sum) / B  with logits[i,j] = -kl[i,j]/T
  kl[i,j] = hneg[j] - Li[i] . Q[j],  Q = softmax(c_j), hneg[j] = sum Q log Q.
  Per-row (i) constant shifts cancel in log-softmax, so log_softmax(c_i) is
  never needed:
    G[i,j] = (c_i[i] . Q[j] - hneg[j]) / T       (logits up to per-row shift)
  With E = exp(c_j) (no max-sub needed for N(0,1) inputs), Z_j = sum_k E[j,k],
  A_j = sum_k E[j,k] c_j[j,k]:
    hneg_j = A_j/Z_j - ln Z_j
    G[i,j] = (S[i,j] + e_j) * s_j,  S = c_i @ E^T,  e_j = Z_j ln Z_j - A_j,
    s_j = 1/(T Z_j)
  loss = sum_i (logsumexp_j G[i,j] - G[i,i]) / B

Sharding: core c takes c_i rows [512c, 512c+512) and a rotated copy of c_j
(np.roll(c_j, -512c, axis=0)) so the diagonal lands at local columns
[0, 512) on every core -> one SPMD NEFF, no per-core addressing. Row
logsumexp is permutation-invariant so the rotation changes nothing else.
Each core returns its scalar partial; the host sums 8 partials / B.
"""

import sys

for _p in ("/opt/trn_rl_repo",):
    if _p not in sys.path:
        sys.path.insert(0, _p)

import numpy as np

import concourse.bass as bass
import concourse.bacc as bacc
import concourse.tile as tile
from concourse import mybir
from concourse import bass_utils

B = 4096
D = 2048
TEMP = 0.5
NCORES = 8
SHARD = B // NCORES  # 512
KT = D // 128  # 16 k partition-tiles
NCH = 8  # 512-wide column chunks
F32 = mybir.dt.float32
F16 = mybir.dt.float16
AF = mybir.ActivationFunctionType
OP = mybir.AluOpType
AX = mybir.AxisListType

NEG_INF = -3.0e38

import os
LOADS_GPSIMD = os.environ.get("K_LOADS_GPSIMD", "0") == "1"
XPOSE_SCALAR = os.environ.get("K_XPOSE_SCALAR", "0") == "1"
PROD_MOD = int(os.environ.get("K_PROD_MOD", "4"))


CSCALE = 4096.0  # power-of-two normalizer keeping W' = E*s*C in fp16 normal range
INV_C = 1.0 / CSCALE


def build_kernel_body(tc, out_ap, ci_ap, cj_ap, eye_ap, reps=1):
    """Emit the kernel IR. out: [1,1] f32; ci: [512,2048] f32;
    cj: [4096,2048] f32 (rotated per-core); eye: [128,128] f32.

    v2: per-j softmax scale is folded into the fp16 rhs operand
    (W' = E * C/(T*Z_j)), the bias row rides the matmul as two fp16 hi/lo
    K-rows, and the row-softmax needs no max subtraction (G in [-11, 27]),
    so ACT consumes PSUM directly: exp(S*2^-12) with free row-sum accum.
    """
    nc = tc.nc

    from contextlib import ExitStack

    with ExitStack() as ctx:
        singles = ctx.enter_context(tc.tile_pool(name="singles", bufs=1))
        xpool = ctx.enter_context(tc.tile_pool(name="xpool", bufs=3))
        epool = ctx.enter_context(tc.tile_pool(name="epool", bufs=3))
        etpool = ctx.enter_context(tc.tile_pool(name="etpool", bufs=5))
        spool = ctx.enter_context(tc.tile_pool(name="spool", bufs=3))
        psS = ctx.enter_context(tc.tile_pool(name="psS", bufs=6, space="PSUM"))
        psX = ctx.enter_context(tc.tile_pool(name="psX", bufs=2, space="PSUM"))

        # constants
        eye32 = singles.tile([128, 128], F32)
        nc.sync.dma_start(out=eye32, in_=eye_ap)
        eye16 = singles.tile([128, 128], F16)
        nc.vector.tensor_copy(out=eye16, in_=eye32)
        ones2 = singles.tile([2, 128], F16)
        nc.vector.memset(ones2, 1.0)
        onesc = singles.tile([128, 1], F32)
        nc.vector.memset(onesc, 1.0)

        # per-j scalar accumulators (col t = j-tile t)
        Zc = singles.tile([128, 32], F32)
        Ac = singles.tile([128, 32], F32)
        sCc = singles.tile([128, 32], F32)
        Zparts = singles.tile([128, 32], F32)  # col = m*8 + n
        Dc = singles.tile([128, 4], F32)
        Zi = singles.tile([128, 4], F32)

        # ci -> fp16 -> transposed [k-part, i] layout
        ciT = singles.tile([128, 4, KT, 128], F16)
        for t in range(4):
            cit = xpool.tile([128, D], F32, tag="xload")
            nc.sync.dma_start(out=cit, in_=ci_ap[128 * t : 128 * (t + 1), :])
            c16 = epool.tile([128, D], F16, tag="estg")
            nc.vector.tensor_copy(out=c16, in_=cit)
            nc.sync.dma_start_transpose(out=ciT[:, t], in_=c16)

        for _rep in range(reps):
            _run_main(tc, ctx, out_ap, cj_ap, locals())


def _run_main(tc, ctx, out_ap, cj_ap, env):
    nc = tc.nc
    singles = env["singles"]; xpool = env["xpool"]; epool = env["epool"]
    etpool = env["etpool"]; spool = env["spool"]; psS = env["psS"]; psX = env["psX"]
    eye32 = env["eye32"]; eye16 = env["eye16"]; ones2 = env["ones2"]; onesc = env["onesc"]
    Zc = env["Zc"]; Ac = env["Ac"]; sCc = env["sCc"]; Zparts = env["Zparts"]
    Dc = env["Dc"]; Zi = env["Zi"]; ciT = env["ciT"]
    if True:
        for n in range(NCH):
            ETc = etpool.tile([128, 4, KT, 128], F16, tag="et")
            for q in range(4):
                t = 4 * n + q
                xt = xpool.tile([128, D], F32, tag="xload")
                (nc.gpsimd if LOADS_GPSIMD else nc.sync).dma_start(
                    out=xt, in_=cj_ap[128 * t : 128 * (t + 1), :]
                )
                es = epool.tile([128, D], F16, tag="estg")
                # E = exp(x); Z_j accumulated for free
                nc.scalar.activation(
                    out=es, in_=xt, func=AF.Exp, accum_out=Zc[:, t : t + 1]
                )
                # A_j = sum_k E*x: product (split DVE/GPSIMD) + DVE reduce
                prod = epool.tile([128, D], F16, tag="prod")
                if t % PROD_MOD == 0:
                    nc.vector.tensor_mul(prod, es, xt)
                else:
                    nc.gpsimd.tensor_mul(prod, es, xt)
                nc.vector.tensor_reduce(
                    out=Ac[:, t : t + 1], in_=prod, axis=AX.X, op=OP.add
                )
                # sC_j = C/(T*Z_j); W' = E*sC in fp16 (normal range)
                nc.vector.tensor_scalar_mul(
                    sCc[:, t : t + 1], Zc[:, t : t + 1], float(TEMP / CSCALE)
                )
                nc.vector.reciprocal(
                    out=sCc[:, t : t + 1], in_=sCc[:, t : t + 1]
                )
                ws = epool.tile([128, D], F16, tag="ws")
                nc.vector.tensor_scalar_mul(ws, es, sCc[:, t : t + 1])
                # W'^T into this chunk's rhs tile (contiguous 3D dest)
                (nc.scalar if XPOSE_SCALAR else nc.sync).dma_start_transpose(
                    out=ETc[:, q], in_=ws
                )

            # per-chunk bias row: b' = (lnZ - A/Z)*(C/T) = lnZ*(C/T) - A*sC
            z4 = Zc[:, 4 * n : 4 * n + 4]
            a4 = Ac[:, 4 * n : 4 * n + 4]
            lnz = spool.tile([128, 4], F32, tag="lnz")
            nc.scalar.activation(out=lnz, in_=z4, func=AF.Ln)
            bp = spool.tile([128, 4], F32, tag="bp")
            nc.vector.tensor_mul(bp, a4, sCc[:, 4 * n : 4 * n + 4])
            lnzs = spool.tile([128, 4], F32, tag="lnzs")
            nc.vector.tensor_scalar_mul(lnzs, lnz, float(CSCALE / TEMP))
            nc.vector.tensor_sub(bp, lnzs, bp)
            # split bias into fp16 hi+lo (keeps fp32 accuracy in the matmul)
            e2 = spool.tile([128, 4, 2], F16, tag="e2")
            nc.vector.tensor_copy(out=e2[:, :, 0], in_=bp)
            nc.vector.tensor_sub(e2[:, :, 1], bp, e2[:, :, 0])
            # transpose per q and collect into one [2, 512] rhs row pair
            e2row = spool.tile([2, 512], F16, tag="e2row")
            for q in range(4):
                e2q_ps = psX.tile([2, 128], F16, tag="xp", bufs=1, name=f"e2ps{n}_{q}")
                nc.tensor.transpose(e2q_ps, e2[:, q, :], eye16)
                nc.vector.tensor_copy(
                    out=e2row[:, 128 * q : 128 * (q + 1)], in_=e2q_ps
                )

            # main matmuls; ACT consumes PSUM directly (exp + row-sum accum)
            for m in range(4):
                S_ps = psS.tile([128, 512], F32, tag="s")
                for kt in range(KT):
                    nc.tensor.matmul(
                        S_ps,
                        ciT[:, m, kt, :],
                        ETc[:, :, kt, :],
                        start=(kt == 0),
                        stop=False,
                    )
                nc.tensor.matmul(S_ps, ones2, e2row, start=False, stop=True)
                if n == 0:
                    junk = spool.tile([128, 128], F32, tag="junk")
                    nc.vector.tensor_mul(
                        junk, S_ps[:, 128 * m : 128 * (m + 1)], eye32
                    )
                    nc.vector.tensor_reduce(
                        out=Dc[:, m : m + 1], in_=junk, axis=AX.X, op=OP.add
                    )
                expj = spool.tile([128, 512], F16, tag="expj", bufs=2)
                nc.scalar.activation(
                    out=expj,
                    in_=S_ps,
                    func=AF.Exp,
                    scale=float(INV_C),
                    accum_out=Zparts[:, 8 * m + n : 8 * m + n + 1],
                )

        # lse_i = ln(sum_n Zparts); loss terms = lse - diag*2^-12
        Zp = Zparts.rearrange("p (m n) -> p m n", n=8)
        nc.vector.tensor_reduce(out=Zi, in_=Zp, axis=AX.X, op=OP.add)
        lnzi = spool.tile([128, 4], F32, tag="lnzi")
        nc.scalar.activation(out=lnzi, in_=Zi, func=AF.Ln)
        gd = spool.tile([128, 4], F32, tag="gd")
        nc.vector.tensor_scalar_mul(gd, Dc, float(INV_C))
        terms = spool.tile([128, 4], F32, tag="terms")
        nc.vector.tensor_sub(terms, lnzi, gd)
        part_ps = psX.tile([1, 4], F32, tag="xp", bufs=1)
        nc.tensor.matmul(part_ps, onesc, terms, start=True, stop=True)
        part = spool.tile([1, 4], F32, tag="part")
        nc.vector.tensor_copy(out=part, in_=part_ps)
        res = spool.tile([1, 1], F32, tag="res")
        nc.vector.reduce_sum(out=res, in_=part, axis=AX.X)
        nc.sync.dma_start(out=out_ap, in_=res)


_NC_CACHE = {}


def build_nc(reps=1):
    key = ("nc", reps)
    if key in _NC_CACHE:
        return _NC_CACHE[key]
    nc = bacc.Bacc("TRN2", target_bir_lowering=False, debug=False)
    ci = nc.dram_tensor("ci", [SHARD, D], F32, kind="ExternalInput").ap()
    cj = nc.dram_tensor("cj", [B, D], F32, kind="ExternalInput").ap()
    eye = nc.dram_tensor("eye", [128, 128], F32, kind="ExternalInput").ap()
    out = nc.dram_tensor("out", [1, 1], F32, kind="ExternalOutput").ap()
    with tile.TileContext(nc) as tc:
        build_kernel_body(tc, out, ci, cj, eye, reps=reps)
    nc.compile()
    _NC_CACHE[key] = nc
    return nc


def make_in_maps(c_i, c_j):
    eye = np.eye(128, dtype=np.float32)
    in_maps = []
    for c in range(NCORES):
        in_maps.append(
            {
                "ci": np.ascontiguousarray(c_i[SHARD * c : SHARD * (c + 1)]),
                "cj": np.ascontiguousarray(np.roll(c_j, -SHARD * c, axis=0)),
                "eye": eye,
            }
        )
    return in_maps


def kernel(c_i, c_j, **kwargs):
    c_i = np.ascontiguousarray(np.asarray(c_i, dtype=np.float32))
    c_j = np.ascontiguousarray(np.asarray(c_j, dtype=np.float32))
    nc = build_nc()
    in_maps = make_in_maps(c_i, c_j)
    res = bass_utils.run_bass_kernel_spmd(
        nc, in_maps, core_ids=list(range(NCORES))
    )
    total = np.float64(0.0)
    for r in res.results:
        total += np.float64(r["out"][0, 0])
    return np.float32(total / B).reshape(())



# revision 13
# speedup vs baseline: 1.8477x; 1.8477x over previous
"""ClusterKLLoss Trainium2 kernel — 8 NeuronCores, j-stripe data-parallel.

Math (from the reference):
  loss·B = sum_i lse_j(G[i,j]) - sum_i G[i,i]
  G[i,j] = (c_i[i]·Q_j - hneg_j)/T,  Q_j = E_j/Z_j,  E = exp(c_j),
  Z_j = sum_k E[j,k],  A_j = sum_k E[j,k]·c_j[j,k],  hneg_j = A_j/Z_j - ln Z_j.

Sharding: core c owns j-stripe [512c, 512c+512). It computes E, Z, A for its
stripe, forms W^T = E^T in fp8, and computes S^T[j,i] against the full
transposed c_i (fp8 input, moving operand) with fp8 DoubleRow matmuls (2x PE
rate, stationary = local E^T tiles). The per-j softmax scale s_j = 1/(T·Z_j)
and bias b_j = lnZ_j/T - A_j·s_j ride the Exp activation's per-partition
scale/bias (S^T has j on partitions). exp tiles accumulate over j-tiles into
Texp[i] (vector adds), are column-summed via one-hot-stationary matmuls, and
an AllReduce(add) across the 8 cores combines per-i partial sums + diagonal
partials; every core computes the identical final scalar on device.

The diagonal G_ii (i in own stripe) is computed position-independently from
natural-layout fp16 tiles via a fused multiply-reduce:
diag_S[i] = sum_k cid[i,k]·E[i,k], G_ii = diag_S·s_i + b_i.

Emission is software-pipelined (P0 P1 M0 P2 M1 P3 M2 M3); ciT lives in 8
per-chunk tiles whose loads are all issued up front across the sync, gpsimd
and scalar DMA queues, so matmuls gate only on their own chunk.
"""

import os
import sys

for _p in ("/opt/trn_rl_repo",):
    if _p not in sys.path:
        sys.path.insert(0, _p)

import numpy as np
import ml_dtypes

import concourse.bass as bass
import concourse.bacc as bacc
import concourse.tile as tile
from concourse import mybir
from concourse import bass_utils

B = 4096
D = 2048
TEMP = 0.5
NCORES = 8
SHARD = B // NCORES  # 512
KT = D // 128  # 16 k partition-tiles
KP = KT // 2  # 8 DoubleRow k-pairs
JT = SHARD // 128  # 4 j-tiles per stripe
ICH = B // 512  # 8 i-chunks of 512

F32 = mybir.dt.float32
F16 = mybir.dt.float16
BF16 = mybir.dt.bfloat16
F8 = mybir.dt.float8e4
AF = mybir.ActivationFunctionType
OP = mybir.AluOpType
AX = mybir.AxisListType
DR = mybir.MatmulPerfMode.DoubleRow

DEBUG_OUT = os.environ.get("K_DEBUG_OUT", "0") == "1"
HOST_COMBINE = os.environ.get("K_HOST_COMBINE", "0") == "1"
USE_TTR = os.environ.get("K_TTR", "0") == "1"
USE_MIXADD = os.environ.get("K_MIXADD", "0") == "1"


def build_kernel_body(tc, out_ap, cit8_ap, cj16_ap, cid16_ap, oh_ap, dbg=None,
                      host_combine=False):
    nc = tc.nc
    from contextlib import ExitStack

    with ExitStack() as ctx:
        singles = ctx.enter_context(tc.tile_pool(name="singles", bufs=1))
        xpool = ctx.enter_context(tc.tile_pool(name="xpool", bufs=3))
        epool = ctx.enter_context(tc.tile_pool(name="epool", bufs=2))
        ppool = ctx.enter_context(tc.tile_pool(name="ppool", bufs=2))
        etpool = ctx.enter_context(tc.tile_pool(name="etpool", bufs=3))
        e32pool = ctx.enter_context(tc.tile_pool(name="e32pool", bufs=3))
        spool = ctx.enter_context(tc.tile_pool(name="spool", bufs=4))
        psS = ctx.enter_context(tc.tile_pool(name="psS", bufs=6, space="PSUM"))
        psE = ctx.enter_context(tc.tile_pool(name="psE", bufs=1, space="PSUM"))
        dram = ctx.enter_context(tc.tile_pool(name="dram", bufs=1, space="DRAM"))

        # full transposed c_i in fp8, one tile per 512-i chunk so matmuls gate
        # only on their own chunk's DMA. Chunks 0-3 on the sync queue, 4-5 on
        # gpsimd (issued before the stripe loads below land), 6-7 on the
        # scalar queue (issued inside P0/P1 after each transpose).
        ciTs = [
            singles.tile([128, KT, 512], F8, name=f"ciT{c}") for c in range(ICH)
        ]
        for c in range(4):
            nc.sync.dma_start(
                out=ciTs[c], in_=cit8_ap[:, :, 512 * c : 512 * (c + 1)]
            )

        oh = singles.tile([128, 8, 8], BF16)
        nc.sync.dma_start(out=oh, in_=oh_ap)
        onesf = singles.tile([128, 1], F32)
        nc.vector.memset(onesf, 1.0)
        ones8 = singles.tile([8, 1], F32)
        nc.vector.memset(ones8, 1.0)

        Z = singles.tile([128, JT], F32)
        A = singles.tile([128, JT], F32)
        sj = singles.tile([128, JT], F32)
        bj = singles.tile([128, JT], F32)
        draw = singles.tile([128, JT], F32)
        Texp = singles.tile([128, B], F32)
        Texbf = singles.tile([128, B], BF16)
        ET8s = [None] * JT

        def phase_p(jt):
            xt = xpool.tile([128, D], F16, tag="xt")
            nc.gpsimd.dma_start(out=xt, in_=cj16_ap[128 * jt : 128 * (jt + 1), :])
            cdt = xpool.tile([128, D], F16, tag="xt")
            nc.gpsimd.dma_start(out=cdt, in_=cid16_ap[128 * jt : 128 * (jt + 1), :])
            if jt == 1:
                for c in (4, 5):
                    nc.gpsimd.dma_start(
                        out=ciTs[c], in_=cit8_ap[:, :, 512 * c : 512 * (c + 1)]
                    )
            es = epool.tile([128, D], F16, tag="es")
            nc.scalar.activation(
                out=es, in_=xt, func=AF.Exp, accum_out=Z[:, jt : jt + 1]
            )
            # W^T = E^T (fp8) for this j-tile: [kpart, ktile, j]
            ET16 = etpool.tile([128, KT, 128], F16, tag="et16")
            nc.scalar.dma_start_transpose(out=ET16, in_=es)
            if jt < 2:  # ciT chunks 6, 7 ride the scalar queue
                c = 6 + jt
                nc.scalar.dma_start(
                    out=ciTs[c], in_=cit8_ap[:, :, 512 * c : 512 * (c + 1)]
                )
            ET8 = etpool.tile([128, KT, 128], F8, tag="et8")
            nc.vector.tensor_copy(out=ET8, in_=ET16)
            ET8s[jt] = ET8
            # s_j = 1/(T*Z); b_j = lnZ/T - A*s_j
            lnz = spool.tile([128, 1], F32, tag="lnz")
            nc.scalar.activation(out=lnz, in_=Z[:, jt : jt + 1], func=AF.Ln)
            nc.vector.tensor_scalar_mul(sj[:, jt : jt + 1], Z[:, jt : jt + 1], TEMP)
            nc.vector.reciprocal(out=sj[:, jt : jt + 1], in_=sj[:, jt : jt + 1])
            # A = sum E*x and diag raw = sum cid*E
            prod = ppool.tile([128, D], F16, tag="prod")
            if USE_TTR:
                nc.vector.tensor_tensor_reduce(
                    out=prod, in0=es, in1=xt, scale=1.0, scalar=0.0,
                    op0=OP.mult, op1=OP.add, accum_out=A[:, jt : jt + 1],
                )
            else:
                nc.vector.tensor_mul(prod, es, xt)
                nc.vector.tensor_reduce(
                    out=A[:, jt : jt + 1], in_=prod, axis=AX.X, op=OP.add
                )
            tmp = spool.tile([128, 1], F32, tag="tmp")
            nc.vector.tensor_mul(tmp, A[:, jt : jt + 1], sj[:, jt : jt + 1])
            nc.vector.tensor_scalar_mul(bj[:, jt : jt + 1], lnz, 1.0 / TEMP)
            nc.vector.tensor_sub(bj[:, jt : jt + 1], bj[:, jt : jt + 1], tmp)
            dprod = ppool.tile([128, D], F16, tag="dprod")
            if USE_TTR:
                nc.vector.tensor_tensor_reduce(
                    out=dprod, in0=cdt, in1=es, scale=1.0, scalar=0.0,
                    op0=OP.mult, op1=OP.add, accum_out=draw[:, jt : jt + 1],
                )
            else:
                nc.vector.tensor_mul(dprod, cdt, es)
                nc.vector.tensor_reduce(
                    out=draw[:, jt : jt + 1], in_=dprod, axis=AX.X, op=OP.add
                )

        def phase_m(jt):
            # S^T[j, i] over full i, fp8 DoubleRow, in 2 psum half-sweeps
            ET8 = ET8s[jt]
            for h in range(2):
                S = [
                    psS.tile([128, 512], F32, tag="s", name=f"S{jt}_{h}_{q}")
                    for q in range(4)
                ]
                for kp in range(KP):
                    for q in range(4):
                        c = 4 * h + q
                        nc.tensor.matmul(
                            S[q],
                            ET8[:, 2 * kp : 2 * kp + 2, :],
                            ciTs[c][:, 2 * kp : 2 * kp + 2, :],
                            start=(kp == 0),
                            stop=(kp == KP - 1),
                            perf_mode=DR,
                        )
                for q in range(4):
                    c = 4 * h + q
                    sl = slice(512 * c, 512 * (c + 1))
                    if jt == 0:
                        nc.scalar.activation(
                            out=Texp[:, sl], in_=S[q], func=AF.Exp,
                            scale=sj[:, jt : jt + 1], bias=bj[:, jt : jt + 1],
                        )
                    else:
                        e32 = e32pool.tile([128, 512], F32, tag="e32")
                        nc.scalar.activation(
                            out=e32, in_=S[q], func=AF.Exp,
                            scale=sj[:, jt : jt + 1], bias=bj[:, jt : jt + 1],
                        )
                        if jt < JT - 1 or not USE_MIXADD:
                            nc.vector.tensor_add(Texp[:, sl], Texp[:, sl], e32)
                        else:  # last pass writes the bf16 colsum operand
                            nc.vector.tensor_add(Texbf[:, sl], Texp[:, sl], e32)

        # software pipeline: P0 P1 M0 P2 M1 P3 M2 M3
        phase_p(0)
        phase_p(1)
        phase_m(0)
        phase_p(2)
        phase_m(1)
        phase_p(3)

        # diagonal early: G_ii = draw*s + b summed over own stripe
        gd = spool.tile([128, JT], F32, tag="gd")
        nc.vector.tensor_mul(gd, draw, sj)
        nc.vector.tensor_add(gd, gd, bj)
        dsum = spool.tile([128, 1], F32, tag="dsum")
        nc.vector.tensor_reduce(out=dsum, in_=gd, axis=AX.X, op=OP.add)
        dps = psE.tile([1, 1], F32, tag="e")
        nc.tensor.matmul(dps, onesf, dsum, start=True, stop=True)
        dsc = spool.tile([1, 1], F32, tag="dsc")
        nc.vector.tensor_copy(out=dsc, in_=dps)

        phase_m(2)
        phase_m(3)

        if dbg is not None:
            zab = spool.tile([128, 5 * JT], F32, tag="zab")
            nc.vector.tensor_copy(out=zab[:, 0:JT], in_=Z)
            nc.vector.tensor_copy(out=zab[:, JT : 2 * JT], in_=A)
            nc.vector.tensor_copy(out=zab[:, 2 * JT : 3 * JT], in_=sj)
            nc.vector.tensor_copy(out=zab[:, 3 * JT : 4 * JT], in_=bj)
            nc.vector.tensor_copy(out=zab[:, 4 * JT : 5 * JT], in_=draw)
            nc.sync.dma_start(out=dbg["zab"], in_=zab)
            nc.sync.dma_start(out=dbg["texp"], in_=Texp)

        # column sums of Texbf via one-hot stationaries -> [8, 512]
        if not USE_MIXADD:
            nc.vector.tensor_copy(out=Texbf, in_=Texp)
        cs = psE.tile([8, 512], F32, tag="e")
        for r in range(8):
            nc.tensor.matmul(
                cs,
                oh[:, r],
                Texbf[:, 512 * r : 512 * (r + 1)],
                start=(r == 0),
                stop=(r == 7),
            )
        cssb = spool.tile([8, 513], F32, tag="cssb")
        nc.vector.memset(cssb, 0.0)
        nc.vector.tensor_copy(out=cssb[:, 0:512], in_=cs)
        nc.vector.tensor_copy(out=cssb[0:1, 512:513], in_=dsc)

        if host_combine:
            nc.sync.dma_start(out=out_ap, in_=cssb)
            return

        # AllReduce partial rowsums + diag partial across the 8 cores
        bin_ = dram.tile([8, 513], F32)
        bout = dram.tile([8, 513], F32)
        nc.gpsimd.dma_start(bin_, cssb)
        nc.gpsimd.collective_compute(
            "AllReduce",
            OP.add,
            replica_groups=[list(range(NCORES))],
            ins=[bin_.opt()],
            outs=[bout.opt()],
        )
        arsb = spool.tile([8, 513], F32, tag="arsb")
        nc.gpsimd.dma_start(arsb, bout)
        if dbg is not None:
            nc.sync.dma_start(out=dbg["cssb"], in_=cssb)
            nc.sync.dma_start(out=dbg["arsb"], in_=arsb)

        # loss*B = sum_i ln(T_i) - sum_i G_ii  (identical on every core)
        lnt = spool.tile([8, 512], F32, tag="lnt")
        lnacc = spool.tile([8, 1], F32, tag="lnacc")
        nc.scalar.activation(
            out=lnt, in_=arsb[:, 0:512], func=AF.Ln, accum_out=lnacc
        )
        tps = psE.tile([1, 1], F32, tag="e")
        nc.tensor.matmul(tps, ones8, lnacc, start=True, stop=True)
        tsb = spool.tile([1, 1], F32, tag="tsb")
        nc.vector.tensor_copy(out=tsb, in_=tps)
        res = spool.tile([1, 1], F32, tag="res")
        nc.vector.tensor_sub(res, tsb, arsb[0:1, 512:513])
        nc.sync.dma_start(out=out_ap, in_=res)


_NC_CACHE = {}


def build_nc():
    key = ("nc", HOST_COMBINE)
    if key in _NC_CACHE:
        return _NC_CACHE[key]
    nc = bacc.Bacc(
        "TRN2", target_bir_lowering=False, debug=False, num_devices=NCORES
    )
    cit8 = nc.dram_tensor("cit8", [128, KT, B], F8, kind="ExternalInput").ap()
    cj16 = nc.dram_tensor("cj16", [SHARD, D], F16, kind="ExternalInput").ap()
    cid16 = nc.dram_tensor("cid16", [SHARD, D], F16, kind="ExternalInput").ap()
    oh = nc.dram_tensor("oh", [128, 8, 8], BF16, kind="ExternalInput").ap()
    out_shape = [8, 513] if HOST_COMBINE else [1, 1]
    out = nc.dram_tensor("out", out_shape, F32, kind="ExternalOutput").ap()
    dbg = None
    if DEBUG_OUT:
        dbg = {
            "zab": nc.dram_tensor("d_zab", [128, 5 * JT], F32, kind="ExternalOutput").ap(),
            "texp": nc.dram_tensor("d_texp", [128, B], F32, kind="ExternalOutput").ap(),
            "cssb": nc.dram_tensor("d_cssb", [8, 513], F32, kind="ExternalOutput").ap(),
            "arsb": nc.dram_tensor("d_arsb", [8, 513], F32, kind="ExternalOutput").ap(),
        }
    with tile.TileContext(nc) as tc:
        build_kernel_body(tc, out, cit8, cj16, cid16, oh, dbg=dbg,
                          host_combine=HOST_COMBINE)
    nc.compile()
    _NC_CACHE[key] = nc
    return nc


def make_in_maps(c_i, c_j):
    c_i = np.ascontiguousarray(np.asarray(c_i, dtype=np.float32))
    c_j = np.ascontiguousarray(np.asarray(c_j, dtype=np.float32))
    ci8 = c_i.astype(ml_dtypes.float8_e4m3)
    # cit8[p, kt, i] = c_i[i, kt*128 + p]
    cit8 = np.ascontiguousarray(ci8.T.reshape(KT, 128, B).transpose(1, 0, 2))
    ci16 = c_i.astype(np.float16)
    cj16 = c_j.astype(np.float16)
    oh = np.zeros((128, 8, 8), ml_dtypes.bfloat16)
    for r in range(8):
        oh[:, r, r] = 1.0
    in_maps = []
    for c in range(NCORES):
        in_maps.append(
            {
                "cit8": cit8,
                "cj16": np.ascontiguousarray(cj16[SHARD * c : SHARD * (c + 1)]),
                "cid16": np.ascontiguousarray(ci16[SHARD * c : SHARD * (c + 1)]),
                "oh": oh,
            }
        )
    return in_maps


def kernel(c_i, c_j, **kwargs):
    nc = build_nc()
    in_maps = make_in_maps(c_i, c_j)
    res = bass_utils.run_bass_kernel_spmd(
        nc, in_maps, core_ids=list(range(NCORES))
    )
    if HOST_COMBINE:
        acc = np.zeros((8, 513), np.float64)
        for r in res.results:
            acc += r["out"].astype(np.float64)
        lossB = np.log(acc[:, 0:512]).sum() - acc[0, 512]
        return np.float32(lossB / B).reshape(())
    return np.float32(np.float64(res.results[0]["out"][0, 0]) / B).reshape(())


# revision 18
# speedup vs baseline: 2.0063x; 1.0858x over previous
"""ClusterKLLoss Trainium2 kernel — 8 NeuronCores, j-stripe data-parallel.

Math (from the reference):
  loss·B = sum_i lse_j(G[i,j]) - sum_i G[i,i]
  G[i,j] = (c_i[i]·Q_j - hneg_j)/T,  Q_j = E_j/Z_j,  E = exp(c_j),
  Z_j = sum_k E[j,k],  A_j = sum_k E[j,k]·c_j[j,k],  hneg_j = A_j/Z_j - ln Z_j.

Sharding: core c owns j-stripe [512c, 512c+512). It computes E, Z, A for its
stripe, forms W^T = E^T in fp8, and computes S^T[j,i] against the full
transposed c_i (fp8 input, moving operand) with fp8 DoubleRow matmuls (2x PE
rate, stationary = local E^T tiles). The per-j softmax scale s_j = 1/(T·Z_j)
and bias b_j = lnZ_j/T - A_j·s_j ride the Exp activation's per-partition
scale/bias (S^T has j on partitions). exp tiles accumulate over j-tiles into
Texp[i] (vector adds), are column-summed via one-hot-stationary matmuls, and
an AllReduce(add) across the 8 cores combines per-i partial sums + diagonal
partials; every core computes the identical final scalar on device.

The diagonal G_ii (i in own stripe) is computed position-independently from
natural-layout fp16 tiles via a fused multiply-reduce:
diag_S[i] = sum_k cid[i,k]·E[i,k], G_ii = diag_S·s_i + b_i.

Emission is software-pipelined (P0 P1 M0 P2 M1 P3 M2 M3); ciT lives in 8
per-chunk tiles whose loads are all issued up front across the sync, gpsimd
and scalar DMA queues, so matmuls gate only on their own chunk.
"""

import os
import sys

for _p in ("/opt/trn_rl_repo",):
    if _p not in sys.path:
        sys.path.insert(0, _p)

import numpy as np
import ml_dtypes

import concourse.bass as bass
import concourse.bacc as bacc
import concourse.tile as tile
from concourse import mybir
from concourse import bass_utils

B = 4096
D = 2048
TEMP = 0.5
NCORES = 8
SHARD = B // NCORES  # 512
KT = D // 128  # 16 k partition-tiles
KP = KT // 2  # 8 DoubleRow k-pairs
JT = SHARD // 128  # 4 j-tiles per stripe
ICH = B // 512  # 8 i-chunks of 512

F32 = mybir.dt.float32
F16 = mybir.dt.float16
BF16 = mybir.dt.bfloat16
F8 = mybir.dt.float8e4
AF = mybir.ActivationFunctionType
OP = mybir.AluOpType
AX = mybir.AxisListType
DR = mybir.MatmulPerfMode.DoubleRow

DEBUG_OUT = os.environ.get("K_DEBUG_OUT", "0") == "1"
HOST_COMBINE = os.environ.get("K_HOST_COMBINE", "0") == "1"
USE_TTR = os.environ.get("K_TTR", "0") == "1"
USE_MIXADD = os.environ.get("K_MIXADD", "0") == "1"


def build_kernel_body(tc, out_ap, cit8_ap, cj16_ap, cid16_ap, oh_ap, dbg=None,
                      host_combine=False):
    nc = tc.nc
    from contextlib import ExitStack

    with ExitStack() as ctx:
        singles = ctx.enter_context(tc.tile_pool(name="singles", bufs=1))
        xpool = ctx.enter_context(tc.tile_pool(name="xpool", bufs=8))
        epool = ctx.enter_context(tc.tile_pool(name="epool", bufs=2))
        ppool = ctx.enter_context(tc.tile_pool(name="ppool", bufs=2))
        etpool = ctx.enter_context(tc.tile_pool(name="etpool", bufs=3))
        e32pool = ctx.enter_context(tc.tile_pool(name="e32pool", bufs=3))
        spool = ctx.enter_context(tc.tile_pool(name="spool", bufs=4))
        psS = ctx.enter_context(tc.tile_pool(name="psS", bufs=6, space="PSUM"))
        psE = ctx.enter_context(tc.tile_pool(name="psE", bufs=1, space="PSUM"))
        dram = ctx.enter_context(tc.tile_pool(name="dram", bufs=1, space="DRAM"))

        # All stripe loads go first on the gpsimd queue (the critical exp ->
        # transpose -> fp8 chain hangs off xt0), then the last two ciT chunks.
        # ciT chunks 0-5 stream on the sync queue; the scalar queue carries
        # only the transposes so they never wait behind megabyte loads.
        xts = [None] * JT
        cdts = [None] * JT
        for jt in range(JT):
            xt = xpool.tile([128, D], F16, tag="xt", name=f"xt{jt}")
            nc.gpsimd.dma_start(out=xt, in_=cj16_ap[128 * jt : 128 * (jt + 1), :])
            cdt = xpool.tile([128, D], F16, tag="xt", name=f"cdt{jt}")
            nc.gpsimd.dma_start(out=cdt, in_=cid16_ap[128 * jt : 128 * (jt + 1), :])
            xts[jt] = xt
            cdts[jt] = cdt

        ciTs = [
            singles.tile([128, KT, 512], F8, name=f"ciT{c}") for c in range(ICH)
        ]
        for c in range(6):
            nc.sync.dma_start(
                out=ciTs[c], in_=cit8_ap[:, :, 512 * c : 512 * (c + 1)]
            )
        for c in (6, 7):
            nc.gpsimd.dma_start(
                out=ciTs[c], in_=cit8_ap[:, :, 512 * c : 512 * (c + 1)]
            )

        oh = singles.tile([128, 8, 8], BF16)
        nc.sync.dma_start(out=oh, in_=oh_ap)
        onesf = singles.tile([128, 1], F32)
        nc.vector.memset(onesf, 1.0)
        ones8 = singles.tile([8, 1], F32)
        nc.vector.memset(ones8, 1.0)

        Z = singles.tile([128, JT], F32)
        A = singles.tile([128, JT], F32)
        sj = singles.tile([128, JT], F32)
        bj = singles.tile([128, JT], F32)
        draw = singles.tile([128, JT], F32)
        Texp = singles.tile([128, B], F32)
        Texbf = singles.tile([128, B], BF16)
        ET8s = [None] * JT

        def phase_p(jt):
            xt = xts[jt]
            cdt = cdts[jt]
            es = epool.tile([128, D], F16, tag="es")
            nc.scalar.activation(
                out=es, in_=xt, func=AF.Exp, accum_out=Z[:, jt : jt + 1]
            )
            # W^T = E^T (fp8) for this j-tile: [kpart, ktile, j]
            ET16 = etpool.tile([128, KT, 128], F16, tag="et16")
            nc.scalar.dma_start_transpose(out=ET16, in_=es)
            ET8 = etpool.tile([128, KT, 128], F8, tag="et8")
            nc.vector.tensor_copy(out=ET8, in_=ET16)
            ET8s[jt] = ET8
            # s_j = 1/(T*Z); b_j = lnZ/T - A*s_j
            lnz = spool.tile([128, 1], F32, tag="lnz")
            nc.scalar.activation(out=lnz, in_=Z[:, jt : jt + 1], func=AF.Ln)
            nc.vector.tensor_scalar_mul(sj[:, jt : jt + 1], Z[:, jt : jt + 1], TEMP)
            nc.vector.reciprocal(out=sj[:, jt : jt + 1], in_=sj[:, jt : jt + 1])
            # A = sum E*x and diag raw = sum cid*E
            prod = ppool.tile([128, D], F16, tag="prod")
            if USE_TTR:
                nc.vector.tensor_tensor_reduce(
                    out=prod, in0=es, in1=xt, scale=1.0, scalar=0.0,
                    op0=OP.mult, op1=OP.add, accum_out=A[:, jt : jt + 1],
                )
            else:
                nc.vector.tensor_mul(prod, es, xt)
                nc.vector.tensor_reduce(
                    out=A[:, jt : jt + 1], in_=prod, axis=AX.X, op=OP.add
                )
            tmp = spool.tile([128, 1], F32, tag="tmp")
            nc.vector.tensor_mul(tmp, A[:, jt : jt + 1], sj[:, jt : jt + 1])
            nc.vector.tensor_scalar_mul(bj[:, jt : jt + 1], lnz, 1.0 / TEMP)
            nc.vector.tensor_sub(bj[:, jt : jt + 1], bj[:, jt : jt + 1], tmp)
            dprod = ppool.tile([128, D], F16, tag="dprod")
            if USE_TTR:
                nc.vector.tensor_tensor_reduce(
                    out=dprod, in0=cdt, in1=es, scale=1.0, scalar=0.0,
                    op0=OP.mult, op1=OP.add, accum_out=draw[:, jt : jt + 1],
                )
            else:
                nc.vector.tensor_mul(dprod, cdt, es)
                nc.vector.tensor_reduce(
                    out=draw[:, jt : jt + 1], in_=dprod, axis=AX.X, op=OP.add
                )

        def phase_m(jt):
            # S^T[j, i] over full i, fp8 DoubleRow, in 2 psum half-sweeps
            ET8 = ET8s[jt]
            for h in range(2):
                S = [
                    psS.tile([128, 512], F32, tag="s", name=f"S{jt}_{h}_{q}")
                    for q in range(4)
                ]
                for kp in range(KP):
                    for q in range(4):
                        c = 4 * h + q
                        nc.tensor.matmul(
                            S[q],
                            ET8[:, 2 * kp : 2 * kp + 2, :],
                            ciTs[c][:, 2 * kp : 2 * kp + 2, :],
                            start=(kp == 0),
                            stop=(kp == KP - 1),
                            perf_mode=DR,
                        )
                for q in range(4):
                    c = 4 * h + q
                    sl = slice(512 * c, 512 * (c + 1))
                    if jt == 0:
                        nc.scalar.activation(
                            out=Texp[:, sl], in_=S[q], func=AF.Exp,
                            scale=sj[:, jt : jt + 1], bias=bj[:, jt : jt + 1],
                        )
                    else:
                        e32 = e32pool.tile([128, 512], F32, tag="e32")
                        nc.scalar.activation(
                            out=e32, in_=S[q], func=AF.Exp,
                            scale=sj[:, jt : jt + 1], bias=bj[:, jt : jt + 1],
                        )
                        if jt < JT - 1 or not USE_MIXADD:
                            nc.vector.tensor_add(Texp[:, sl], Texp[:, sl], e32)
                            if jt == JT - 1:  # bf16 colsum operand, per chunk
                                nc.vector.tensor_copy(
                                    out=Texbf[:, sl], in_=Texp[:, sl]
                                )
                        else:  # last pass writes the bf16 colsum operand
                            nc.vector.tensor_add(Texbf[:, sl], Texp[:, sl], e32)

        # software pipeline: P0 P1 M0 P2 M1 P3 M2 M3
        phase_p(0)
        phase_p(1)
        phase_m(0)
        phase_p(2)
        phase_m(1)
        phase_p(3)

        # diagonal early: G_ii = draw*s + b summed over own stripe
        gd = spool.tile([128, JT], F32, tag="gd")
        nc.vector.tensor_mul(gd, draw, sj)
        nc.vector.tensor_add(gd, gd, bj)
        dsum = spool.tile([128, 1], F32, tag="dsum")
        nc.vector.tensor_reduce(out=dsum, in_=gd, axis=AX.X, op=OP.add)
        dps = psE.tile([1, 1], F32, tag="e")
        nc.tensor.matmul(dps, onesf, dsum, start=True, stop=True)
        dsc = spool.tile([1, 1], F32, tag="dsc")
        nc.vector.tensor_copy(out=dsc, in_=dps)

        phase_m(2)
        phase_m(3)

        if dbg is not None:
            zab = spool.tile([128, 5 * JT], F32, tag="zab")
            nc.vector.tensor_copy(out=zab[:, 0:JT], in_=Z)
            nc.vector.tensor_copy(out=zab[:, JT : 2 * JT], in_=A)
            nc.vector.tensor_copy(out=zab[:, 2 * JT : 3 * JT], in_=sj)
            nc.vector.tensor_copy(out=zab[:, 3 * JT : 4 * JT], in_=bj)
            nc.vector.tensor_copy(out=zab[:, 4 * JT : 5 * JT], in_=draw)
            nc.sync.dma_start(out=dbg["zab"], in_=zab)
            nc.sync.dma_start(out=dbg["texp"], in_=Texp)

        # column sums of Texbf via one-hot stationaries -> [8, 512]
        cs = psE.tile([8, 512], F32, tag="e")
        for r in range(8):
            nc.tensor.matmul(
                cs,
                oh[:, r],
                Texbf[:, 512 * r : 512 * (r + 1)],
                start=(r == 0),
                stop=(r == 7),
            )
        cssb = spool.tile([8, 513], F32, tag="cssb")
        nc.vector.memset(cssb, 0.0)
        nc.vector.tensor_copy(out=cssb[:, 0:512], in_=cs)
        nc.vector.tensor_copy(out=cssb[0:1, 512:513], in_=dsc)

        if host_combine:
            nc.sync.dma_start(out=out_ap, in_=cssb)
            return

        # AllReduce partial rowsums + diag partial across the 8 cores
        bin_ = dram.tile([8, 513], F32)
        bout = dram.tile([8, 513], F32)
        nc.gpsimd.dma_start(bin_, cssb)
        nc.gpsimd.collective_compute(
            "AllReduce",
            OP.add,
            replica_groups=[list(range(NCORES))],
            ins=[bin_.opt()],
            outs=[bout.opt()],
        )
        arsb = spool.tile([8, 513], F32, tag="arsb")
        nc.gpsimd.dma_start(arsb, bout)
        if dbg is not None:
            nc.sync.dma_start(out=dbg["cssb"], in_=cssb)
            nc.sync.dma_start(out=dbg["arsb"], in_=arsb)

        # loss*B = sum_i ln(T_i) - sum_i G_ii  (identical on every core)
        lnt = spool.tile([8, 512], F32, tag="lnt")
        lnacc = spool.tile([8, 1], F32, tag="lnacc")
        nc.scalar.activation(
            out=lnt, in_=arsb[:, 0:512], func=AF.Ln, accum_out=lnacc
        )
        tps = psE.tile([1, 1], F32, tag="e")
        nc.tensor.matmul(tps, ones8, lnacc, start=True, stop=True)
        tsb = spool.tile([1, 1], F32, tag="tsb")
        nc.vector.tensor_copy(out=tsb, in_=tps)
        res = spool.tile([1, 1], F32, tag="res")
        nc.vector.tensor_sub(res, tsb, arsb[0:1, 512:513])
        nc.sync.dma_start(out=out_ap, in_=res)


_NC_CACHE = {}


def build_nc():
    key = ("nc", HOST_COMBINE)
    if key in _NC_CACHE:
        return _NC_CACHE[key]
    nc = bacc.Bacc(
        "TRN2", target_bir_lowering=False, debug=False, num_devices=NCORES
    )
    cit8 = nc.dram_tensor("cit8", [128, KT, B], F8, kind="ExternalInput").ap()
    cj16 = nc.dram_tensor("cj16", [SHARD, D], F16, kind="ExternalInput").ap()
    cid16 = nc.dram_tensor("cid16", [SHARD, D], F16, kind="ExternalInput").ap()
    oh = nc.dram_tensor("oh", [128, 8, 8], BF16, kind="ExternalInput").ap()
    out_shape = [8, 513] if HOST_COMBINE else [1, 1]
    out = nc.dram_tensor("out", out_shape, F32, kind="ExternalOutput").ap()
    dbg = None
    if DEBUG_OUT:
        dbg = {
            "zab": nc.dram_tensor("d_zab", [128, 5 * JT], F32, kind="ExternalOutput").ap(),
            "texp": nc.dram_tensor("d_texp", [128, B], F32, kind="ExternalOutput").ap(),
            "cssb": nc.dram_tensor("d_cssb", [8, 513], F32, kind="ExternalOutput").ap(),
            "arsb": nc.dram_tensor("d_arsb", [8, 513], F32, kind="ExternalOutput").ap(),
        }
    with tile.TileContext(nc) as tc:
        build_kernel_body(tc, out, cit8, cj16, cid16, oh, dbg=dbg,
                          host_combine=HOST_COMBINE)
    nc.compile()
    _NC_CACHE[key] = nc
    return nc


def make_in_maps(c_i, c_j):
    c_i = np.ascontiguousarray(np.asarray(c_i, dtype=np.float32))
    c_j = np.ascontiguousarray(np.asarray(c_j, dtype=np.float32))
    ci8 = c_i.astype(ml_dtypes.float8_e4m3)
    # cit8[p, kt, i] = c_i[i, kt*128 + p]
    cit8 = np.ascontiguousarray(ci8.T.reshape(KT, 128, B).transpose(1, 0, 2))
    ci16 = c_i.astype(np.float16)
    cj16 = c_j.astype(np.float16)
    oh = np.zeros((128, 8, 8), ml_dtypes.bfloat16)
    for r in range(8):
        oh[:, r, r] = 1.0
    in_maps = []
    for c in range(NCORES):
        in_maps.append(
            {
                "cit8": cit8,
                "cj16": np.ascontiguousarray(cj16[SHARD * c : SHARD * (c + 1)]),
                "cid16": np.ascontiguousarray(ci16[SHARD * c : SHARD * (c + 1)]),
                "oh": oh,
            }
        )
    return in_maps


def kernel(c_i, c_j, **kwargs):
    nc = build_nc()
    in_maps = make_in_maps(c_i, c_j)
    res = bass_utils.run_bass_kernel_spmd(
        nc, in_maps, core_ids=list(range(NCORES))
    )
    if HOST_COMBINE:
        acc = np.zeros((8, 513), np.float64)
        for r in res.results:
            acc += r["out"].astype(np.float64)
        lossB = np.log(acc[:, 0:512]).sum() - acc[0, 512]
        return np.float32(lossB / B).reshape(())
    return np.float32(np.float64(res.results[0]["out"][0, 0]) / B).reshape(())
